# revision 37
# baseline (speedup 1.0000x reference)
"""GNN message-passing classifier on 8 Trainium2 NeuronCores (Bass/Tile).

Full inputs in, full outputs out. Internally:
  - nodes (and edges by destination) are partitioned across the 8 cores,
  - small weights replicated,
  - transformed features all-gathered between layers (split in two halves
    so the first half overlaps the tail of the producing pass),
  - per-graph readout finished with an AllReduce.

Math (algebraically identical to the reference):
  deg  = indegree(dst)            (host-precomputed structural metadata) [N]
  a    = where(deg>0, segsum(deg[src],dst)/deg, deg)  [N]
  p    = relu(a*W1 + b1) @ W2                         [N,128]   (W2 pushed
         through the (linear) mean-aggregation of layer 2)
  q    = segsum(p[src], dst)                          [N,128]
  h2   = relu(where(deg>0, q/deg, p) + b2)            [N,128]
  out  = (segmean(h2, graph_ids)) @ Wc + bc           [G,2]

Segment sums run as one-hot matmuls accumulating in fp32 PSUM. Since
edges arrive sorted by destination, up to MEMB edges sharing a
destination are packed into one "slot", so one one-hot matrix S serves
MEMB gathered operands (MEMB matmuls) — the DVE one-hot build cost drops
by ~MEMB x. Missing slot members point at rows that are guaranteed zero
(a zero tile appended to the first node half, and the tail padding of
the second half). bf16 datapath for the one-hot matmuls and the p table
(exact for 0/1 one-hots and small-integer degree sums); the a->h1->p
value chain and the q normalization stay fp32.
"""

import math
import os

import ml_dtypes
import numpy as np

import concourse.bass as bass
import concourse.bacc as bacc
import concourse.mybir as mybir
import concourse.tile as tile

F32 = mybir.dt.float32
BF16 = mybir.dt.bfloat16
I32 = mybir.dt.int32
AX = mybir.AluOpType
AF = mybir.ActivationFunctionType
NPBF = ml_dtypes.bfloat16

# -------- fixed problem config (hardcoded; kernel.py must be self-contained)
FULL_CFG = dict(N=100000, E=1600000, G=128, H=256, NC=8)
MEMB = 1          # edges packed per destination slot

# last run results (exec_time_ns etc.) for the local test harness
LAST_RESULTS = None


# --------------------------------------------------------------------------
# host-side sharding prep (pure index shuffling / padding)
# --------------------------------------------------------------------------
def host_prep(src, dst, graph_ids, cfg):
    N, NC = cfg["N"], cfg["NC"]
    NPC = N // NC                      # 12500 nodes per core
    TR = math.ceil(NPC / 128)          # 98 real tiles
    HT = TR // 2 + 1                   # 50 layout tiles in half A (incl zero)
    ZT = HT - 1                        # zero tile index 49
    TL = TR + 1                        # 99 layout tiles
    HSH = HT * 128                     # 6400 rows in half A
    SH = TL * 128                      # 12672 layout rows per core
    BSZ = SH - HSH                     # 6272 rows in half B
    ACUT = ZT * 128                    # 6272: nodes below go to half A
    ZROW_A = ACUT                      # first zero row (core 0) in table A
    ZROW_B = NPC - ACUT                # first zero row (core 0) in table B

    src = np.asarray(src).astype(np.int64)
    dst = np.asarray(dst).astype(np.int64)
    gid = np.asarray(graph_ids).astype(np.int64)

    order = np.argsort(dst, kind="stable")
    ds = dst[order]
    ss = src[order]
    dc = ds // NPC                     # destination core
    dl = ds % NPC                      # local node id
    dly = np.where(dl >= ACUT, dl + 128, dl)   # layout row (skip zero tile)
    dtile = dly // 128                 # layout tile (never ZT)
    dst_p = (dly % 128).astype(np.int32)
    sl = ss % NPC
    src_hi = sl >= ACUT                # source in half B
    sc = ss // NPC
    srow = np.where(src_hi, sc * BSZ + (sl - ACUT), sc * HSH + sl).astype(
        np.int64)

    gt = dc * TL + dtile               # global (core,tile) bucket
    gorder = np.argsort(gt * 2 + src_hi, kind="stable")
    ds, ss, dtile, dst_p, src_hi, srow, gt = (
        x[gorder] for x in (ds, ss, dtile, dst_p, src_hi, srow, gt))

    # slot packing: run of edges with equal (bucket, half, dst_p) packs
    # MEMB edges per slot
    key = (gt * 2 + src_hi) * 128 + dst_p
    uniq, start, counts = np.unique(key, return_index=True,
                                    return_counts=True)
    rank = np.arange(len(key)) - np.repeat(start, counts)
    slot_in_run = rank // MEMB
    member = rank % MEMB
    nslots_run = -(-counts // MEMB)

    # slots per (bucket, half)
    bh = uniq // 128                   # bucket*2+half of each run
    nslots_bh = np.zeros(NC * TL * 2, np.int64)
    np.add.at(nslots_bh, bh, nslots_run)
    nA = nslots_bh.reshape(NC, TL, 2)[:, :, 0]
    nB = nslots_bh.reshape(NC, TL, 2)[:, :, 1]
    kA_list = [int(-(-nA[:, t].max() // 128)) for t in range(TL)]
    kB_list = [int(-(-nB[:, t].max() // 128)) for t in range(TL)]
    k_list = [a + b for a, b in zip(kA_list, kB_list)]
    assert k_list[ZT] == 0
    CH = sum(k_list)
    koff = np.concatenate([[0], np.cumsum(k_list)]).astype(int)

    # slot id within (bucket, half): base of own run + slot_in_run
    run_base = np.zeros(len(uniq), np.int64)
    prev = None
    # vectorized exclusive cumsum of nslots_run per bh group
    csl = np.concatenate([[0], np.cumsum(nslots_run)])[:-1]
    grp_first = np.concatenate([[True], bh[1:] != bh[:-1]])
    grp_base = np.repeat(csl[grp_first], np.diff(
        np.concatenate([np.flatnonzero(grp_first), [len(uniq)]])))
    run_base = csl - grp_base
    slot_bh = np.repeat(run_base, counts) + slot_in_run   # slot within (b,h)

    # in-degrees: structural metadata, shipped as input features
    deg = np.bincount(dst, minlength=N).astype(np.float32)
    deg_all = np.zeros((NC, 128, TL), np.float32)
    dega = np.zeros(NC * HSH, np.float32)
    degb = np.zeros(NC * BSZ, np.float32)
    for c in range(NC):
        dcore = deg[c * NPC : (c + 1) * NPC]
        la = np.arange(NPC)
        lay = np.where(la >= ACUT, la + 128, la)
        deg_all[c, lay % 128, lay // 128] = dcore
        dega[c * HSH : c * HSH + ACUT] = dcore[:ACUT]
        degb[c * BSZ : c * BSZ + (NPC - ACUT)] = dcore[ACUT:]

    # fill tables
    dl4 = np.full((NC, 128, CH), -1.0, np.float32)
    src4 = np.zeros((MEMB, NC, 128, CH), np.int32)
    ebucket = gt                     # per edge
    ehalf = src_hi.astype(np.int64)
    ec = ebucket // TL
    et = ebucket % TL
    ek0 = koff[et] + np.where(ehalf == 1, np.array(kA_list)[et], 0)
    col = ek0 + slot_bh // 128
    p_of = slot_bh % 128
    # default member rows -> zero rows of the matching half (per column)
    for t in range(TL):
        j0 = koff[t]
        kA = kA_list[t]
        k = k_list[t]
        src4[:, :, :, j0 : j0 + kA] = ZROW_A
        src4[:, :, :, j0 + kA : j0 + k] = ZROW_B
    dl4[ec, p_of, col] = dst_p.astype(np.float32)
    src4[member, ec, p_of, col] = srow.astype(np.int32)

    gl = np.full((NC, 128, TL), -1.0, np.float32)
    larr = np.arange(NPC)
    lay = np.where(larr >= ACUT, larr + 128, larr)
    for c in range(NC):
        gl[c, lay % 128, lay // 128] = gid[c * NPC : (c + 1) * NPC].astype(
            np.float32
        )

    # mask for the final (partially padded) tile
    padmask = (np.arange(128) < (NPC - ACUT) % 128).astype(
        np.float32)[:, None]

    return dict(
        NPC=NPC, TL=TL, ZT=ZT, SH=SH, HT=HT, HSH=HSH, BSZ=BSZ, CH=CH,
        k_list=k_list, kA_list=kA_list, koff=koff,
        src4=src4, dl4=dl4, graph_loc=gl,
        deg_all=deg_all, dega=dega[:, None], degb=degb[:, None],
        padmask=padmask,
    )


def host_weights(W1, b1, W2, b2, Wc, bc):
    W1 = np.asarray(W1, np.float32).reshape(256)
    b1 = np.asarray(b1, np.float32).reshape(256)
    W2 = np.asarray(W2, np.float32)
    b2 = np.asarray(b2, np.float32).reshape(128)
    Wc = np.asarray(Wc, np.float32)
    bc = np.asarray(bc, np.float32).reshape(2)
    eye = np.eye(128, dtype=np.float32)
    return dict(
        w1c=np.stack([W1[:128], W1[128:]], axis=1),         # [128,2] f32
        b1c=np.stack([b1[:128], b1[128:]], axis=1),         # [128,2] f32
        W2a=np.ascontiguousarray(W2[:128]),                 # [128,128] f32
        W2b=np.ascontiguousarray(W2[128:]),                 # [128,128] f32
        b2rep=np.tile(b2[None, :], (128, 1)),               # [128,128] f32
        Wc=np.ascontiguousarray(Wc),                        # [128,2] f32
        bcrep=np.tile(bc[None, :], (128, 1)),               # [128,2] f32
        iota_bf=np.tile(np.arange(128, dtype=np.float32)[None, :],
                        (128, 1)).astype(NPBF),
        identf=eye,
    )


# --------------------------------------------------------------------------
# device program
# --------------------------------------------------------------------------
def _build_onehot(nc, sp, iota_ap, dl_sb, j0, k, tag):
    """One-hot chunks S_j [128,128] bf16: S_j[p, d] = (dl4[p,j0+j] == d).

    Two batched DVE is_equal ops (split so the consumer can start on the
    first half while the second builds). Returns list of k APs.
    """
    S = sp.tile([128, k * 128], BF16, tag=tag)
    kh = (k + 1) // 2
    for a, b in ((0, kh), (kh, k)):
        m = b - a
        if m <= 0:
            continue
        S3 = S[:, a * 128 : b * 128].rearrange("p (k d) -> p k d", d=128)
        dl3 = dl_sb[:, j0 + a : j0 + b].unsqueeze(2).to_broadcast([128, m, 128])
        io3 = iota_ap.unsqueeze(1).to_broadcast([128, m, 128])
        nc.vector.tensor_tensor(out=S3[:], in0=dl3, in1=io3, op=AX.is_equal)
    return [S[:, j * 128 : (j + 1) * 128] for j in range(k)]


def build_program(prep, cfg, phases=4):
    NC, G = cfg["NC"], cfg["G"]
    TL, ZT, SH, CH = prep["TL"], prep["ZT"], prep["SH"], prep["CH"]
    HT, HSH, BSZ = prep["HT"], prep["HSH"], prep["BSZ"]
    k_list, kA_list, koff = prep["k_list"], prep["kA_list"], prep["koff"]
    H2 = 128

    nc = bacc.Bacc("TRN2", target_bir_lowering=False, debug=False,
                   num_devices=NC)

    # packed constants: one DMA per dtype
    CBW = CH + TL + 128                   # dl4 gl iota
    CFW = 8 + 4 * 128 + TL + 1            # w1c b1c Wc bcrep identf W2a W2b b2rep deg_all padmask
    d_cb = nc.dram_tensor("constb", [128, CBW], BF16, kind="ExternalInput")
    d_cf = nc.dram_tensor("constf", [128, CFW], F32, kind="ExternalInput")
    d_ci = nc.dram_tensor("consti", [128, MEMB * CH], I32,
                          kind="ExternalInput")
    d_dega = nc.dram_tensor("dega", [NC * HSH, 1], F32, kind="ExternalInput")
    d_degb = nc.dram_tensor("degb", [NC * BSZ, 1], F32, kind="ExternalInput")
    d_out = nc.dram_tensor("out", [128, 2], F32, kind="ExternalOutput")

    with tile.TileContext(nc) as tc:
        with (
            tc.tile_pool(name="const", bufs=1) as cp,
            tc.tile_pool(name="dram", bufs=1, space="DRAM") as dp,
        ):
            # ------- internal DRAM
            p_sh = dp.tile([SH, H2], BF16, tag="p_sh")
            p_full_a = dp.tile([NC * HSH, H2], BF16, tag="p_full_a",
                               addr_space="Shared")
            p_full_b = dp.tile([NC * BSZ, H2], BF16, tag="p_full_b",
                               addr_space="Shared")
            gs_in = dp.tile([128, H2 + 1], F32, tag="gs_in")
            gs_out = dp.tile([128, H2 + 1], F32, tag="gs_out",
                             addr_space="Shared")

            # ------- resident SBUF constants
            CB = cp.tile([128, CBW], BF16, tag="CB")
            CF = cp.tile([128, CFW], F32, tag="CF")
            CI = cp.tile([128, MEMB * CH], I32, tag="CI")
            ob = [0]
            of = [0]

            def _cb(w):
                ap = CB[:, ob[0] : ob[0] + w]
                ob[0] += w
                return ap

            def _cf(w):
                ap = CF[:, of[0] : of[0] + w]
                of[0] += w
                return ap

            dl_sb = _cb(CH)
            iota_sb = _cb(128)
            gl_sb = _cb(TL)
            w1_sb = _cf(2)
            b1_sb = _cf(2)
            Wc_sb = _cf(2)
            bc_sb = _cf(2)
            identf = _cf(128)
            W2a_sb = _cf(128)
            W2b_sb = _cf(128)
            b2_sb = _cf(128)
            deg_all = _cf(TL)
            padmask = _cf(1)

            def src_m(m, lo, hi):
                return CI[:, m * CH + lo : m * CH + hi]

            # per-node-shard stats, one column per tile
            a_all = cp.tile([128, TL], F32, tag="a_all")
            recip_all = cp.tile([128, TL], F32, tag="recip_all")
            mask0_all = cp.tile([128, TL], mybir.dt.uint8, tag="mask0_all")
            # own shard's p kept resident in SBUF (bf16, ~25KB/partition)
            p_own = cp.tile([128, TL * 128], BF16, tag="p_own")

            nc.sync.dma_start(out=CB[:], in_=d_cb[:])
            nc.sync.dma_start(out=CF[:], in_=d_cf[:])
            nc.sync.dma_start(out=CI[:], in_=d_ci[:])
            iota_ap = iota_sb

            # node stats (reciprocal + 2 Newton steps: HW recip is coarse)
            degc_all = cp.tile([128, TL], F32, tag="degc_all")
            rtmp = cp.tile([128, TL], F32, tag="rtmp")
            nc.vector.tensor_scalar(out=degc_all[:], in0=deg_all,
                                    scalar1=1.0, scalar2=None, op0=AX.max)
            nc.vector.reciprocal(out=recip_all[:], in_=degc_all[:])
            for _ in range(2):
                nc.vector.tensor_mul(out=rtmp[:], in0=degc_all[:],
                                     in1=recip_all[:])
                nc.vector.tensor_scalar(out=rtmp[:], in0=rtmp[:],
                                        scalar1=-1.0, scalar2=2.0,
                                        op0=AX.mult, op1=AX.add)
                nc.vector.tensor_mul(out=recip_all[:], in0=recip_all[:],
                                     in1=rtmp[:])
            nc.vector.tensor_scalar(out=mask0_all[:], in0=deg_all,
                                    scalar1=0.0, scalar2=None, op0=AX.is_le)

            if phases == 1:
                dbg = cp.tile([128, 2], F32, tag="dbg")
                nc.vector.tensor_copy(out=dbg[:], in_=deg_all[:, 0:2])
                nc.sync.dma_start(out=d_out[:], in_=dbg[:])

            if phases >= 2:
                # =========== fused pass: a -> p (per destination tile) =====
                with (
                    tc.tile_pool(name="p2s", bufs=3) as sp2,
                    tc.tile_pool(name="p2p", bufs=2, space="PSUM") as pp2,
                ):
                    for t in range(TL):
                        k = k_list[t]
                        kA = kA_list[t]
                        j0 = koff[t]
                        pob = p_own[:, t * 128 : (t + 1) * 128]
                        if t == ZT:
                            nc.vector.memset(pob, 0.0)
                            nc.sync.dma_start(
                                out=p_sh[t * 128 : (t + 1) * 128, :], in_=pob)
                            if t == HT - 1:
                                # collective triggers do not wait for
                                # in-flight DMA writes of their input
                                # (see gs_in note below) — drain first
                                tc.strict_bb_all_engine_barrier()
                                nc.gpsimd.collective_compute(
                                    "AllGather", AX.bypass,
                                    ins=[p_sh[0:HSH, :].opt()],
                                    outs=[p_full_a[:].opt()],
                                    replica_groups=[list(range(NC))],
                                )
                            continue
                        dsrc = sp2.tile([128, MEMB * k], F32, tag="dsrc")
                        for m in range(MEMB):
                            if kA:
                                nc.gpsimd.indirect_dma_start(
                                    out=dsrc[:, m * k : m * k + kA],
                                    out_offset=None,
                                    in_=d_dega[:],
                                    in_offset=bass.IndirectOffsetOnAxis(
                                        ap=src_m(m, j0, j0 + kA), axis=0),
                                )
                            if k > kA:
                                nc.gpsimd.indirect_dma_start(
                                    out=dsrc[:, m * k + kA : (m + 1) * k],
                                    out_offset=None,
                                    in_=d_degb[:],
                                    in_offset=bass.IndirectOffsetOnAxis(
                                        ap=src_m(m, j0 + kA, j0 + k), axis=0),
                                )
                        dsb = sp2.tile([128, k], BF16, tag="dsb")
                        if MEMB == 1:
                            nc.vector.tensor_copy(out=dsb[:], in_=dsrc[:])
                        else:
                            dsum = sp2.tile([128, k], F32, tag="dsum")
                            nc.vector.tensor_add(
                                out=dsum[:], in0=dsrc[:, 0:k],
                                in1=dsrc[:, k : 2 * k])
                            for m in range(2, MEMB):
                                nc.vector.tensor_add(
                                    out=dsum[:], in0=dsum[:],
                                    in1=dsrc[:, m * k : (m + 1) * k])
                            nc.vector.tensor_copy(out=dsb[:], in_=dsum[:])
                        Sl = _build_onehot(nc, sp2, iota_ap, dl_sb, j0, k, "s2")
                        nps = pp2.tile([128, 1], F32, tag="nump", space="PSUM")
                        for j in range(k):
                            nc.tensor.matmul(
                                out=nps[:], lhsT=Sl[j],
                                rhs=dsb[:, j : j + 1], start=(j == 0),
                                stop=(j == k - 1),
                            )
                        if phases == 9 and t == 0:
                            dbg9 = sp2.tile([128, 2], F32, tag="dbg9")
                            c9 = int(os.environ.get("GNN_DBG9_COL", "0"))
                            nc.vector.tensor_copy(
                                out=dbg9[:], in_=dsrc[:, c9 : c9 + 2])
                            nc.sync.dma_start(out=d_out[:], in_=dbg9[:])
                        if phases == 8 and t == 0:
                            dbg8 = sp2.tile([128, 2], F32, tag="dbg8")
                            nc.vector.tensor_copy(
                                out=dbg8[:],
                                in_=nps[:].to_broadcast([128, 2]))
                            nc.sync.dma_start(out=d_out[:], in_=dbg8[:])
                        acol = a_all[:, t : t + 1]
                        nc.vector.tensor_scalar(
                            out=acol, in0=nps[:],
                            scalar1=recip_all[:, t : t + 1], scalar2=None,
                            op0=AX.mult,
                        )
                        nc.vector.copy_predicated(
                            out=acol, mask=mask0_all[:, t : t + 1],
                            data=deg_all[:, t : t + 1])
                        atp = pp2.tile([128, 128], F32, tag="atp",
                                       space="PSUM")
                        nc.tensor.transpose(
                            out=atp[:],
                            in_=acol.to_broadcast([128, 128]),
                            identity=identf,
                        )
                        pps = pp2.tile([128, H2], F32, tag="pps", space="PSUM")
                        for kk, W2_sb in ((0, W2a_sb), (1, W2b_sb)):
                            h1k = sp2.tile([128, 128], F32, tag=f"h1k{kk}")
                            nc.scalar.activation(
                                out=h1k[:], in_=atp[:], func=AF.Relu,
                                bias=b1_sb[:, kk : kk + 1],
                                scale=w1_sb[:, kk : kk + 1],
                            )
                            nc.tensor.matmul(out=pps[:], lhsT=h1k[:],
                                             rhs=W2_sb,
                                             start=(kk == 0), stop=(kk == 1))
                        if t == TL - 1:
                            # zero the 44 padded node slots so table B's
                            # tail provides the zero rows
                            nc.vector.tensor_scalar(
                                out=pob, in0=pps[:], scalar1=padmask,
                                scalar2=None, op0=AX.mult)
                        else:
                            nc.vector.tensor_copy(out=pob, in_=pps[:])
                        nc.sync.dma_start(out=p_sh[t * 128 : (t + 1) * 128, :],
                                          in_=pob)

            if phases == 2:
                dbg = cp.tile([128, 2], F32, tag="dbg")
                nc.vector.tensor_copy(out=dbg[:], in_=a_all[:, 0:2])
                nc.sync.dma_start(out=d_out[:], in_=dbg[:])

            if phases >= 3:
                tc.strict_bb_all_engine_barrier()
                nc.gpsimd.collective_compute(
                    "AllGather", AX.bypass,
                    ins=[p_sh[HSH:SH, :].opt()],
                    outs=[p_full_b[:].opt()],
                    replica_groups=[list(range(NC))],
                )

            if phases == 3:
                dbg = cp.tile([128, 2], F32, tag="dbg")
                dbgb = cp.tile([128, 2], BF16, tag="dbgb")
                dbg_r = int(os.environ.get("GNN_DBG3_ROW", "0"))
                nc.sync.dma_start(
                    out=dbgb[:],
                    in_=p_full_a[dbg_r : dbg_r + 128, 0:2])
                nc.vector.tensor_copy(out=dbg[:], in_=dbgb[:])
                nc.sync.dma_start(out=d_out[:], in_=dbg[:])

            if phases >= 4 and phases not in (8, 9):
                # =========== pass 3: q -> h2 -> graph readout ===========
                with (
                    tc.tile_pool(name="p3s", bufs=3) as sp3,
                    tc.tile_pool(name="p3g",
                                 bufs=int(os.environ.get("GNN_B3", "3"))) as gp3,
                    tc.tile_pool(name="p3p", bufs=3, space="PSUM") as pp3,
                    tc.tile_pool(name="p3a", bufs=1, space="PSUM") as pacc,
                ):
                    gsum = pacc.tile([128, H2 + 1], F32, tag="gsum",
                                     space="PSUM")
                    for t in range(TL):
                        if t == ZT or (phases == 7 and t > 0):
                            continue
                        k = k_list[t]
                        kA = kA_list[t]
                        j0 = koff[t]
                        Gt = gp3.tile([128, MEMB * k * 128], BF16, tag="Gt")
                        for m in range(MEMB):
                            c0 = m * k * 128
                            if kA:
                                nc.gpsimd.indirect_dma_start(
                                    out=Gt[:, c0 : c0 + kA * 128],
                                    out_offset=None,
                                    in_=p_full_a[:],
                                    in_offset=bass.IndirectOffsetOnAxis(
                                        ap=src_m(m, j0, j0 + kA), axis=0),
                                )
                            if k > kA:
                                nc.gpsimd.indirect_dma_start(
                                    out=Gt[:, c0 + kA * 128 : c0 + k * 128],
                                    out_offset=None,
                                    in_=p_full_b[:],
                                    in_offset=bass.IndirectOffsetOnAxis(
                                        ap=src_m(m, j0 + kA, j0 + k), axis=0),
                                )
                        Sl = _build_onehot(nc, sp3, iota_ap, dl_sb, j0, k, "s3")
                        qps = pp3.tile([128, H2], F32, tag="qps", space="PSUM")
                        for j in range(k):
                            for m in range(MEMB):
                                nc.tensor.matmul(
                                    out=qps[:], lhsT=Sl[j],
                                    rhs=Gt[:, (m * k + j) * 128
                                           : (m * k + j + 1) * 128],
                                    start=(j == 0 and m == 0),
                                    stop=(j == k - 1 and m == MEMB - 1),
                                )
                        if phases == 7:
                            dbg7 = sp3.tile([128, 2], F32, tag="dbg7")
                            nc.vector.tensor_copy(out=dbg7[:],
                                                  in_=qps[:, 0:2])
                            nc.sync.dma_start(out=d_out[:], in_=dbg7[:])
                            continue
                        qn = sp3.tile([128, H2], F32, tag="qn")
                        nc.vector.tensor_scalar(
                            out=qn[:], in0=qps[:],
                            scalar1=recip_all[:, t : t + 1], scalar2=None,
                            op0=AX.mult,
                        )
                        pof = sp3.tile([128, H2], F32, tag="pof")
                        nc.vector.tensor_copy(
                            out=pof[:], in_=p_own[:, t * 128 : (t + 1) * 128])
                        nc.vector.copy_predicated(
                            out=qn[:],
                            mask=mask0_all[:, t : t + 1].to_broadcast([128, H2]),
                            data=pof[:],
                        )
                        h2 = sp3.tile([128, H2 + 1], BF16, tag="h2")
                        nc.vector.tensor_add(out=qn[:], in0=qn[:], in1=b2_sb)
                        nc.scalar.activation(out=h2[:, 0:H2], in_=qn[:],
                                             func=AF.Relu)
                        nc.vector.memset(h2[:, H2 : H2 + 1], 1.0)
                        goh = sp3.tile([128, 128], BF16, tag="goh")
                        nc.vector.tensor_tensor(
                            out=goh[:],
                            in0=gl_sb[:, t : t + 1].to_broadcast([128, 128]),
                            in1=iota_ap, op=AX.is_equal,
                        )
                        nc.tensor.matmul(out=gsum[:], lhsT=goh[:], rhs=h2[:],
                                         start=(t == 0), stop=(t == TL - 1))

                    if phases != 7:
                        gs_sb = sp3.tile([128, H2 + 1], F32, tag="gs_sb")
                        nc.vector.tensor_copy(out=gs_sb[:], in_=gsum[:])
                        if phases == 6:
                            nc.sync.dma_start(out=d_out[:], in_=gs_sb[:, 0:2])
                        else:
                            nc.sync.dma_start(out=gs_in[:], in_=gs_sb[:])

            if phases >= 4 and phases not in (6, 7, 8, 9):
                # gs_in is written by a DMA just above; the collective
                # trigger does not reliably wait for that write (graph-0
                # row garbage) — force completion first.
                tc.strict_bb_all_engine_barrier()
                nc.gpsimd.collective_compute(
                    "AllReduce", AX.add,
                    ins=[gs_in[:].opt()], outs=[gs_out[:].opt()],
                    replica_groups=[list(range(NC))],
                )

                # =========== final readout ===========
                with (
                    tc.tile_pool(name="fs", bufs=1) as fs,
                    tc.tile_pool(name="fp", bufs=1, space="PSUM") as fp,
                ):
                    gs2 = fs.tile([128, H2 + 1], F32, tag="gs2")
                    nc.sync.dma_start(out=gs2[:], in_=gs_out[:])
                    rcnt = fs.tile([128, 1], F32, tag="rcnt")
                    cntc = fs.tile([128, 1], F32, tag="cntc")
                    ctmp = fs.tile([128, 1], F32, tag="ctmp")
                    nc.vector.tensor_scalar(out=cntc[:],
                                            in0=gs2[:, H2 : H2 + 1],
                                            scalar1=1.0, scalar2=None,
                                            op0=AX.max)
                    nc.vector.reciprocal(out=rcnt[:], in_=cntc[:])
                    for _ in range(2):
                        nc.vector.tensor_mul(out=ctmp[:], in0=cntc[:],
                                             in1=rcnt[:])
                        nc.vector.tensor_scalar(out=ctmp[:], in0=ctmp[:],
                                                scalar1=-1.0, scalar2=2.0,
                                                op0=AX.mult, op1=AX.add)
                        nc.vector.tensor_mul(out=rcnt[:], in0=rcnt[:],
                                             in1=ctmp[:])
                    gr = fs.tile([128, H2], F32, tag="gr")
                    nc.vector.tensor_scalar(out=gr[:], in0=gs2[:, 0:H2],
                                            scalar1=rcnt[:], scalar2=None,
                                            op0=AX.mult)
                    grtp = fp.tile([128, H2], F32, tag="grtp", space="PSUM")
                    nc.tensor.transpose(out=grtp[:], in_=gr[:],
                                        identity=identf)
                    grt = fs.tile([128, H2], F32, tag="grt")
                    nc.vector.tensor_copy(out=grt[:], in_=grtp[:])
                    lps = fp.tile([128, 2], F32, tag="lps", space="PSUM")
                    nc.tensor.matmul(out=lps[:], lhsT=grt[:], rhs=Wc_sb,
                                     start=True, stop=True)
                    ologit = fs.tile([128, 2], F32, tag="ologit")
                    nc.vector.tensor_add(out=ologit[:], in0=lps[:], in1=bc_sb)
                    nc.sync.dma_start(out=d_out[:], in_=ologit[:])

    nc.compile()
    return nc


def make_in_maps(prep, wts, cfg):
    NC = cfg["NC"]
    maps = []
    for c in range(NC):
        constb = np.concatenate([
            prep["dl4"][c].astype(NPBF), wts["iota_bf"],
            prep["graph_loc"][c].astype(NPBF),
        ], axis=1)
        constf = np.concatenate([
            wts["w1c"], wts["b1c"], wts["Wc"], wts["bcrep"], wts["identf"],
            wts["W2a"], wts["W2b"], wts["b2rep"], prep["deg_all"][c],
            prep["padmask"],
        ], axis=1).astype(np.float32)
        consti = np.concatenate(
            [prep["src4"][m][c] for m in range(MEMB)], axis=1).astype(np.int32)
        maps.append(dict(constb=np.ascontiguousarray(constb),
                         constf=np.ascontiguousarray(constf),
                         consti=np.ascontiguousarray(consti),
                         dega=prep["dega"], degb=prep["degb"]))
    return maps


# --------------------------------------------------------------------------
# entry point
# --------------------------------------------------------------------------
def kernel(src, dst, graph_ids, W1, b1, W2, b2, Wc, bc):
    global LAST_RESULTS
    from concourse.bass_utils import run_bass_kernel_spmd

    cfg = FULL_CFG
    prep = host_prep(src, dst, graph_ids, cfg)
    wts = host_weights(W1, b1, W2, b2, Wc, bc)
    nc = build_program(prep, cfg)
    in_maps = make_in_maps(prep, wts, cfg)
    trace = bool(os.environ.get("GNN_TRACE"))
    res = run_bass_kernel_spmd(
        nc, in_maps, core_ids=list(range(cfg["NC"])), trace=trace,
    )
    LAST_RESULTS = res
    out = np.asarray(res.results[0]["out"])[: cfg["G"]]
    return out.astype(np.float32)


# revision 49
# speedup vs baseline: 1.0714x; 1.0714x over previous
"""GNN message-passing classifier on 8 Trainium2 NeuronCores (Bass/Tile).

Full inputs in, full outputs out. Internally:
  - nodes (and edges by destination) are partitioned across the 8 cores,
  - small weights replicated,
  - transformed features all-gathered between layers (split in two halves
    so the first half overlaps the tail of the producing pass),
  - per-graph readout finished with an AllReduce.

Math (algebraically identical to the reference):
  deg  = indegree(dst)            (host-precomputed structural metadata) [N]
  a    = where(deg>0, segsum(deg[src],dst)/deg, deg)  [N]
  p    = relu(a*W1 + b1) @ W2                         [N,128]   (W2 pushed
         through the (linear) mean-aggregation of layer 2)
  q    = segsum(p[src], dst)                          [N,128]
  h2   = relu(where(deg>0, q/deg, p) + b2)            [N,128]
  out  = (segmean(h2, graph_ids)) @ Wc + bc           [G,2]

Segment sums run as one-hot matmuls accumulating in fp32 PSUM. Since
edges arrive sorted by destination, up to MEMB edges sharing a
destination are packed into one "slot", so one one-hot matrix S serves
MEMB gathered operands (MEMB matmuls) — the DVE one-hot build cost drops
by ~MEMB x. Missing slot members point at rows that are guaranteed zero
(a zero tile appended to the first node half, and the tail padding of
the second half). bf16 datapath for the one-hot matmuls and the p table
(exact for 0/1 one-hots and small-integer degree sums); the a->h1->p
value chain and the q normalization stay fp32.
"""

import math
import os

import ml_dtypes
import numpy as np

import concourse.bass as bass
import concourse.bacc as bacc
import concourse.mybir as mybir
import concourse.tile as tile

F32 = mybir.dt.float32
BF16 = mybir.dt.bfloat16
I32 = mybir.dt.int32
AX = mybir.AluOpType
AF = mybir.ActivationFunctionType
NPBF = ml_dtypes.bfloat16

# -------- fixed problem config (hardcoded; kernel.py must be self-contained)
FULL_CFG = dict(N=100000, E=1600000, G=128, H=256, NC=8)
MEMB = 1          # edges packed per destination slot

# last run results (exec_time_ns etc.) for the local test harness
LAST_RESULTS = None


# --------------------------------------------------------------------------
# host-side sharding prep (pure index shuffling / padding)
# --------------------------------------------------------------------------
def host_prep(src, dst, graph_ids, cfg):
    N, NC = cfg["N"], cfg["NC"]
    NPC = N // NC                      # 12500 nodes per core
    TR = math.ceil(NPC / 128)          # 98 real tiles
    ZT = TR // 2                       # zero tile index 49
    TL = TR + 1                        # 99 layout tiles
    HT = TL                            # single AllGather: "half A" = all
    SH = TL * 128                      # 12672 layout rows per core
    HSH = HT * 128
    BSZ = SH - HSH                     # 0: no half B
    ACUT = ZT * 128                    # 6272: layout rows skip zero tile
    ZROW_A = ACUT                      # first zero row (core 0)
    ZROW_B = 0

    src = np.asarray(src).astype(np.int64)
    dst = np.asarray(dst).astype(np.int64)
    gid = np.asarray(graph_ids).astype(np.int64)

    order = np.argsort(dst, kind="stable")
    ds = dst[order]
    ss = src[order]
    dc = ds // NPC                     # destination core
    dl = ds % NPC                      # local node id
    dly = np.where(dl >= ACUT, dl + 128, dl)   # layout row (skip zero tile)
    dtile = dly // 128                 # layout tile (never ZT)
    dst_p = (dly % 128).astype(np.int32)
    sl = ss % NPC
    src_hi = np.zeros(len(ss), np.bool_)   # single table: no half B
    sc = ss // NPC
    sly = np.where(sl >= ACUT, sl + 128, sl)   # source layout row
    srow = (sc * SH + sly).astype(np.int64)

    gt = dc * TL + dtile               # global (core,tile) bucket
    gorder = np.argsort(gt * 2 + src_hi, kind="stable")
    ds, ss, dtile, dst_p, src_hi, srow, gt = (
        x[gorder] for x in (ds, ss, dtile, dst_p, src_hi, srow, gt))

    # slot packing: run of edges with equal (bucket, half, dst_p) packs
    # MEMB edges per slot
    key = (gt * 2 + src_hi) * 128 + dst_p
    uniq, start, counts = np.unique(key, return_index=True,
                                    return_counts=True)
    rank = np.arange(len(key)) - np.repeat(start, counts)
    slot_in_run = rank // MEMB
    member = rank % MEMB
    nslots_run = -(-counts // MEMB)

    # slots per (bucket, half)
    bh = uniq // 128                   # bucket*2+half of each run
    nslots_bh = np.zeros(NC * TL * 2, np.int64)
    np.add.at(nslots_bh, bh, nslots_run)
    nA = nslots_bh.reshape(NC, TL, 2)[:, :, 0]
    nB = nslots_bh.reshape(NC, TL, 2)[:, :, 1]
    kA_list = [int(-(-nA[:, t].max() // 128)) for t in range(TL)]
    kB_list = [int(-(-nB[:, t].max() // 128)) for t in range(TL)]
    k_list = [a + b for a, b in zip(kA_list, kB_list)]
    assert k_list[ZT] == 0
    CH = sum(k_list)
    koff = np.concatenate([[0], np.cumsum(k_list)]).astype(int)

    # slot id within (bucket, half): base of own run + slot_in_run
    run_base = np.zeros(len(uniq), np.int64)
    prev = None
    # vectorized exclusive cumsum of nslots_run per bh group
    csl = np.concatenate([[0], np.cumsum(nslots_run)])[:-1]
    grp_first = np.concatenate([[True], bh[1:] != bh[:-1]])
    grp_base = np.repeat(csl[grp_first], np.diff(
        np.concatenate([np.flatnonzero(grp_first), [len(uniq)]])))
    run_base = csl - grp_base
    slot_bh = np.repeat(run_base, counts) + slot_in_run   # slot within (b,h)

    # in-degrees: structural metadata, shipped as input features
    deg = np.bincount(dst, minlength=N).astype(np.float32)
    deg_all = np.zeros((NC, 128, TL), np.float32)
    dega = np.zeros(NC * SH, np.float32)
    degb = np.zeros(max(1, NC * BSZ), np.float32)
    for c in range(NC):
        dcore = deg[c * NPC : (c + 1) * NPC]
        la = np.arange(NPC)
        lay = np.where(la >= ACUT, la + 128, la)
        deg_all[c, lay % 128, lay // 128] = dcore
        dega[c * SH + lay] = dcore

    # fill tables
    dl4 = np.full((NC, 128, CH), -1.0, np.float32)
    src4 = np.zeros((MEMB, NC, 128, CH), np.int32)
    ebucket = gt                     # per edge
    ehalf = src_hi.astype(np.int64)
    ec = ebucket // TL
    et = ebucket % TL
    ek0 = koff[et] + np.where(ehalf == 1, np.array(kA_list)[et], 0)
    col = ek0 + slot_bh // 128
    p_of = slot_bh % 128
    # default member rows -> zero rows of the matching half (per column)
    for t in range(TL):
        j0 = koff[t]
        kA = kA_list[t]
        k = k_list[t]
        src4[:, :, :, j0 : j0 + kA] = ZROW_A
        src4[:, :, :, j0 + kA : j0 + k] = ZROW_B
    dl4[ec, p_of, col] = dst_p.astype(np.float32)
    src4[member, ec, p_of, col] = srow.astype(np.int32)

    gl = np.full((NC, 128, TL), -1.0, np.float32)
    larr = np.arange(NPC)
    lay = np.where(larr >= ACUT, larr + 128, larr)
    for c in range(NC):
        gl[c, lay % 128, lay // 128] = gid[c * NPC : (c + 1) * NPC].astype(
            np.float32
        )

    # mask for the final (partially padded) tile
    padmask = (np.arange(128) < (NPC - ACUT) % 128).astype(
        np.float32)[:, None]

    return dict(
        NPC=NPC, TL=TL, ZT=ZT, SH=SH, HT=HT, HSH=HSH, BSZ=BSZ, CH=CH,
        k_list=k_list, kA_list=kA_list, koff=koff,
        src4=src4, dl4=dl4, graph_loc=gl,
        deg_all=deg_all, dega=dega[:, None], degb=degb[:, None],
        padmask=padmask,
    )


def host_weights(W1, b1, W2, b2, Wc, bc):
    W1 = np.asarray(W1, np.float32).reshape(256)
    b1 = np.asarray(b1, np.float32).reshape(256)
    W2 = np.asarray(W2, np.float32)
    b2 = np.asarray(b2, np.float32).reshape(128)
    Wc = np.asarray(Wc, np.float32)
    bc = np.asarray(bc, np.float32).reshape(2)
    eye = np.eye(128, dtype=np.float32)
    return dict(
        w1c=np.stack([W1[:128], W1[128:]], axis=1),         # [128,2] f32
        b1c=np.stack([b1[:128], b1[128:]], axis=1),         # [128,2] f32
        W2a=np.ascontiguousarray(W2[:128]),                 # [128,128] f32
        W2b=np.ascontiguousarray(W2[128:]),                 # [128,128] f32
        b2rep=np.tile(b2[None, :], (128, 1)),               # [128,128] f32
        Wc=np.ascontiguousarray(Wc),                        # [128,2] f32
        bcrep=np.tile(bc[None, :], (128, 1)),               # [128,2] f32
        iota_bf=np.tile(np.arange(128, dtype=np.float32)[None, :],
                        (128, 1)).astype(NPBF),
        identf=eye,
    )


# --------------------------------------------------------------------------
# device program
# --------------------------------------------------------------------------
def _build_onehot(nc, sp, iota_ap, dl_sb, j0, k, tag):
    """One-hot chunks S_j [128,128] bf16: S_j[p, d] = (dl4[p,j0+j] == d).

    Two batched DVE is_equal ops (split so the consumer can start on the
    first half while the second builds). Returns list of k APs.
    """
    S = sp.tile([128, k * 128], BF16, tag=tag)
    kh = (k + 1) // 2
    for a, b in ((0, kh), (kh, k)):
        m = b - a
        if m <= 0:
            continue
        S3 = S[:, a * 128 : b * 128].rearrange("p (k d) -> p k d", d=128)
        dl3 = dl_sb[:, j0 + a : j0 + b].unsqueeze(2).to_broadcast([128, m, 128])
        io3 = iota_ap.unsqueeze(1).to_broadcast([128, m, 128])
        nc.vector.tensor_tensor(out=S3[:], in0=dl3, in1=io3, op=AX.is_equal)
    return [S[:, j * 128 : (j + 1) * 128] for j in range(k)]


def build_program(prep, cfg, phases=4):
    NC, G = cfg["NC"], cfg["G"]
    TL, ZT, SH, CH = prep["TL"], prep["ZT"], prep["SH"], prep["CH"]
    HT, HSH, BSZ = prep["HT"], prep["HSH"], prep["BSZ"]
    k_list, kA_list, koff = prep["k_list"], prep["kA_list"], prep["koff"]
    H2 = 128

    nc = bacc.Bacc("TRN2", target_bir_lowering=False, debug=False,
                   num_devices=NC)

    # packed constants: one DMA per dtype
    CBW = CH + TL + 128                   # dl4 gl iota
    CFW = 8 + 4 * 128 + TL + 1            # w1c b1c Wc bcrep identf W2a W2b b2rep deg_all padmask
    d_cb = nc.dram_tensor("constb", [128, CBW], BF16, kind="ExternalInput")
    d_cf = nc.dram_tensor("constf", [128, CFW], F32, kind="ExternalInput")
    d_ci = nc.dram_tensor("consti", [128, MEMB * CH], I32,
                          kind="ExternalInput")
    d_dega = nc.dram_tensor("dega", [NC * HSH, 1], F32, kind="ExternalInput")
    d_degb = nc.dram_tensor("degb", [max(1, NC * BSZ), 1], F32,
                            kind="ExternalInput")
    d_out = nc.dram_tensor("out", [128, 2], F32, kind="ExternalOutput")

    with tile.TileContext(nc) as tc:
        with (
            tc.tile_pool(name="const", bufs=1) as cp,
            tc.tile_pool(name="dram", bufs=1, space="DRAM") as dp,
        ):
            # ------- internal DRAM
            p_sh = dp.tile([SH, H2], BF16, tag="p_sh")
            p_full_a = dp.tile([NC * HSH, H2], BF16, tag="p_full_a",
                               addr_space="Shared")
            p_full_b = (dp.tile([NC * BSZ, H2], BF16, tag="p_full_b",
                                addr_space="Shared") if BSZ else None)
            gs_in = dp.tile([128, H2 + 1], F32, tag="gs_in")
            gs_out = dp.tile([128, H2 + 1], F32, tag="gs_out",
                             addr_space="Shared")

            # ------- resident SBUF constants
            CB = cp.tile([128, CBW], BF16, tag="CB")
            CF = cp.tile([128, CFW], F32, tag="CF")
            CI = cp.tile([128, MEMB * CH], I32, tag="CI")
            ob = [0]
            of = [0]

            def _cb(w):
                ap = CB[:, ob[0] : ob[0] + w]
                ob[0] += w
                return ap

            def _cf(w):
                ap = CF[:, of[0] : of[0] + w]
                of[0] += w
                return ap

            dl_sb = _cb(CH)
            iota_sb = _cb(128)
            gl_sb = _cb(TL)
            w1_sb = _cf(2)
            b1_sb = _cf(2)
            Wc_sb = _cf(2)
            bc_sb = _cf(2)
            identf = _cf(128)
            W2a_sb = _cf(128)
            W2b_sb = _cf(128)
            b2_sb = _cf(128)
            deg_all = _cf(TL)
            padmask = _cf(1)

            def src_m(m, lo, hi):
                return CI[:, m * CH + lo : m * CH + hi]

            # per-node-shard stats, one column per tile
            a_all = cp.tile([128, TL], F32, tag="a_all")
            recip_all = cp.tile([128, TL], F32, tag="recip_all")
            mask0_all = cp.tile([128, TL], mybir.dt.uint8, tag="mask0_all")
            # own shard's p kept resident in SBUF (bf16, ~25KB/partition)
            p_own = cp.tile([128, TL * 128], BF16, tag="p_own")

            nc.sync.dma_start(out=CB[:], in_=d_cb[:])
            nc.sync.dma_start(out=CF[:], in_=d_cf[:])
            nc.sync.dma_start(out=CI[:], in_=d_ci[:])
            iota_ap = iota_sb

            # node stats (reciprocal + 2 Newton steps: HW recip is coarse)
            degc_all = cp.tile([128, TL], F32, tag="degc_all")
            rtmp = cp.tile([128, TL], F32, tag="rtmp")
            nc.vector.tensor_scalar(out=degc_all[:], in0=deg_all,
                                    scalar1=1.0, scalar2=None, op0=AX.max)
            nc.vector.reciprocal(out=recip_all[:], in_=degc_all[:])
            for _ in range(2):
                nc.vector.tensor_mul(out=rtmp[:], in0=degc_all[:],
                                     in1=recip_all[:])
                nc.vector.tensor_scalar(out=rtmp[:], in0=rtmp[:],
                                        scalar1=-1.0, scalar2=2.0,
                                        op0=AX.mult, op1=AX.add)
                nc.vector.tensor_mul(out=recip_all[:], in0=recip_all[:],
                                     in1=rtmp[:])
            nc.vector.tensor_scalar(out=mask0_all[:], in0=deg_all,
                                    scalar1=0.0, scalar2=None, op0=AX.is_le)

            if phases == 1:
                dbg = cp.tile([128, 2], F32, tag="dbg")
                nc.vector.tensor_copy(out=dbg[:], in_=deg_all[:, 0:2])
                nc.sync.dma_start(out=d_out[:], in_=dbg[:])

            if phases >= 2:
                # =========== fused pass: a -> p (per destination tile) =====
                with (
                    tc.tile_pool(name="p2s", bufs=3) as sp2,
                    tc.tile_pool(name="p2n", bufs=2, space="PSUM") as pp2n,
                    tc.tile_pool(name="p2p", bufs=3, space="PSUM") as pp2,
                ):
                    for t in range(TL):
                        k = k_list[t]
                        kA = kA_list[t]
                        j0 = koff[t]
                        pob = p_own[:, t * 128 : (t + 1) * 128]

                        def _maybe_ag_a():
                            if t == HT - 1:
                                # collective triggers do not wait for
                                # in-flight DMA writes of their input
                                # (see gs_in note below) — drain first
                                tc.strict_bb_all_engine_barrier()
                                nc.gpsimd.collective_compute(
                                    "AllGather", AX.bypass,
                                    ins=[p_sh[0:HSH, :].opt()],
                                    outs=[p_full_a[:].opt()],
                                    replica_groups=[list(range(NC))],
                                )

                        if t == ZT:
                            nc.vector.memset(pob, 0.0)
                            nc.sync.dma_start(
                                out=p_sh[t * 128 : (t + 1) * 128, :], in_=pob)
                            _maybe_ag_a()
                            continue
                        dsrc = sp2.tile([128, MEMB * k], F32, tag="dsrc")
                        for m in range(MEMB):
                            if kA:
                                nc.gpsimd.indirect_dma_start(
                                    out=dsrc[:, m * k : m * k + kA],
                                    out_offset=None,
                                    in_=d_dega[:],
                                    in_offset=bass.IndirectOffsetOnAxis(
                                        ap=src_m(m, j0, j0 + kA), axis=0),
                                )
                            if k > kA:
                                nc.gpsimd.indirect_dma_start(
                                    out=dsrc[:, m * k + kA : (m + 1) * k],
                                    out_offset=None,
                                    in_=d_degb[:],
                                    in_offset=bass.IndirectOffsetOnAxis(
                                        ap=src_m(m, j0 + kA, j0 + k), axis=0),
                                )
                        dsb = sp2.tile([128, k], BF16, tag="dsb")
                        if MEMB == 1:
                            nc.vector.tensor_copy(out=dsb[:], in_=dsrc[:])
                        else:
                            dsum = sp2.tile([128, k], F32, tag="dsum")
                            nc.vector.tensor_add(
                                out=dsum[:], in0=dsrc[:, 0:k],
                                in1=dsrc[:, k : 2 * k])
                            for m in range(2, MEMB):
                                nc.vector.tensor_add(
                                    out=dsum[:], in0=dsum[:],
                                    in1=dsrc[:, m * k : (m + 1) * k])
                            nc.vector.tensor_copy(out=dsb[:], in_=dsum[:])
                        Sl = _build_onehot(nc, sp2, iota_ap, dl_sb, j0, k, "s2")
                        nps = pp2n.tile([128, 1], F32, tag="nump",
                                        space="PSUM")
                        for j in range(k):
                            nc.tensor.matmul(
                                out=nps[:], lhsT=Sl[j],
                                rhs=dsb[:, j : j + 1], start=(j == 0),
                                stop=(j == k - 1),
                            )
                        if phases == 9 and t == 0:
                            dbg9 = sp2.tile([128, 2], F32, tag="dbg9")
                            c9 = int(os.environ.get("GNN_DBG9_COL", "0"))
                            nc.vector.tensor_copy(
                                out=dbg9[:], in_=dsrc[:, c9 : c9 + 2])
                            nc.sync.dma_start(out=d_out[:], in_=dbg9[:])
                        if phases == 8 and t == 0:
                            dbg8 = sp2.tile([128, 2], F32, tag="dbg8")
                            nc.vector.tensor_copy(
                                out=dbg8[:],
                                in_=nps[:].to_broadcast([128, 2]))
                            nc.sync.dma_start(out=d_out[:], in_=dbg8[:])
                        acol = a_all[:, t : t + 1]
                        nc.vector.tensor_scalar(
                            out=acol, in0=nps[:],
                            scalar1=recip_all[:, t : t + 1], scalar2=None,
                            op0=AX.mult,
                        )
                        nc.vector.copy_predicated(
                            out=acol, mask=mask0_all[:, t : t + 1],
                            data=deg_all[:, t : t + 1])
                        atp = pp2.tile([128, 128], F32, tag="atp",
                                       space="PSUM")
                        nc.tensor.transpose(
                            out=atp[:],
                            in_=acol.to_broadcast([128, 128]),
                            identity=identf,
                        )
                        pps = pp2.tile([128, H2], F32, tag="pps", space="PSUM")
                        for kk, W2_sb in ((0, W2a_sb), (1, W2b_sb)):
                            h1k = sp2.tile([128, 128], F32, tag=f"h1k{kk}")
                            nc.scalar.activation(
                                out=h1k[:], in_=atp[:], func=AF.Relu,
                                bias=b1_sb[:, kk : kk + 1],
                                scale=w1_sb[:, kk : kk + 1],
                            )
                            nc.tensor.matmul(out=pps[:], lhsT=h1k[:],
                                             rhs=W2_sb,
                                             start=(kk == 0), stop=(kk == 1))
                        if t == TL - 1:
                            # zero the 44 padded node slots so table B's
                            # tail provides the zero rows
                            nc.vector.tensor_scalar(
                                out=pob, in0=pps[:], scalar1=padmask,
                                scalar2=None, op0=AX.mult)
                        else:
                            nc.vector.tensor_copy(out=pob, in_=pps[:])
                        nc.sync.dma_start(out=p_sh[t * 128 : (t + 1) * 128, :],
                                          in_=pob)
                        _maybe_ag_a()

            if phases == 2:
                dbg = cp.tile([128, 2], F32, tag="dbg")
                nc.vector.tensor_copy(out=dbg[:], in_=a_all[:, 0:2])
                nc.sync.dma_start(out=d_out[:], in_=dbg[:])

            if phases >= 3 and HSH < SH:
                tc.strict_bb_all_engine_barrier()
                nc.gpsimd.collective_compute(
                    "AllGather", AX.bypass,
                    ins=[p_sh[HSH:SH, :].opt()],
                    outs=[p_full_b[:].opt()],
                    replica_groups=[list(range(NC))],
                )

            if phases == 3:
                dbg = cp.tile([128, 2], F32, tag="dbg")
                dbgb = cp.tile([128, 2], BF16, tag="dbgb")
                dbg_r = int(os.environ.get("GNN_DBG3_ROW", "0"))
                nc.sync.dma_start(
                    out=dbgb[:],
                    in_=p_full_a[dbg_r : dbg_r + 128, 0:2])
                nc.vector.tensor_copy(out=dbg[:], in_=dbgb[:])
                nc.sync.dma_start(out=d_out[:], in_=dbg[:])

            if phases >= 4 and phases not in (8, 9):
                # =========== pass 3: q -> h2 -> graph readout ===========
                with (
                    tc.tile_pool(name="p3s", bufs=3) as sp3,
                    tc.tile_pool(name="p3g",
                                 bufs=int(os.environ.get("GNN_B3", "3"))) as gp3,
                    tc.tile_pool(name="p3p", bufs=3, space="PSUM") as pp3,
                    tc.tile_pool(name="p3a", bufs=1, space="PSUM") as pacc,
                ):
                    gsum = pacc.tile([128, H2 + 1], F32, tag="gsum",
                                     space="PSUM")
                    for t in range(TL):
                        if t == ZT or (phases == 7 and t > 0):
                            continue
                        k = k_list[t]
                        kA = kA_list[t]
                        j0 = koff[t]
                        Gt = gp3.tile([128, MEMB * k * 128], BF16, tag="Gt")
                        for m in range(MEMB):
                            c0 = m * k * 128
                            if kA:
                                nc.gpsimd.indirect_dma_start(
                                    out=Gt[:, c0 : c0 + kA * 128],
                                    out_offset=None,
                                    in_=p_full_a[:],
                                    in_offset=bass.IndirectOffsetOnAxis(
                                        ap=src_m(m, j0, j0 + kA), axis=0),
                                )
                            if k > kA:
                                nc.gpsimd.indirect_dma_start(
                                    out=Gt[:, c0 + kA * 128 : c0 + k * 128],
                                    out_offset=None,
                                    in_=p_full_b[:],
                                    in_offset=bass.IndirectOffsetOnAxis(
                                        ap=src_m(m, j0 + kA, j0 + k), axis=0),
                                )
                        Sl = _build_onehot(nc, sp3, iota_ap, dl_sb, j0, k, "s3")
                        qps = pp3.tile([128, H2], F32, tag="qps", space="PSUM")
                        for j in range(k):
                            for m in range(MEMB):
                                nc.tensor.matmul(
                                    out=qps[:], lhsT=Sl[j],
                                    rhs=Gt[:, (m * k + j) * 128
                                           : (m * k + j + 1) * 128],
                                    start=(j == 0 and m == 0),
                                    stop=(j == k - 1 and m == MEMB - 1),
                                )
                        if phases == 7:
                            dbg7 = sp3.tile([128, 2], F32, tag="dbg7")
                            nc.vector.tensor_copy(out=dbg7[:],
                                                  in_=qps[:, 0:2])
                            nc.sync.dma_start(out=d_out[:], in_=dbg7[:])
                            continue
                        qn = sp3.tile([128, H2], F32, tag="qn")
                        nc.vector.tensor_scalar(
                            out=qn[:], in0=qps[:],
                            scalar1=recip_all[:, t : t + 1], scalar2=None,
                            op0=AX.mult,
                        )
                        pof = sp3.tile([128, H2], F32, tag="pof")
                        nc.scalar.copy(
                            out=pof[:], in_=p_own[:, t * 128 : (t + 1) * 128])
                        nc.vector.copy_predicated(
                            out=qn[:],
                            mask=mask0_all[:, t : t + 1].to_broadcast([128, H2]),
                            data=pof[:],
                        )
                        h2 = sp3.tile([128, H2 + 1], BF16, tag="h2")
                        nc.vector.tensor_add(out=qn[:], in0=qn[:], in1=b2_sb)
                        nc.scalar.activation(out=h2[:, 0:H2], in_=qn[:],
                                             func=AF.Relu)
                        nc.vector.memset(h2[:, H2 : H2 + 1], 1.0)
                        goh = sp3.tile([128, 128], BF16, tag="goh")
                        nc.vector.tensor_tensor(
                            out=goh[:],
                            in0=gl_sb[:, t : t + 1].to_broadcast([128, 128]),
                            in1=iota_ap, op=AX.is_equal,
                        )
                        nc.tensor.matmul(out=gsum[:], lhsT=goh[:], rhs=h2[:],
                                         start=(t == 0), stop=(t == TL - 1))

                    if phases != 7:
                        gs_sb = sp3.tile([128, H2 + 1], F32, tag="gs_sb")
                        nc.vector.tensor_copy(out=gs_sb[:], in_=gsum[:])
                        if phases == 6:
                            nc.sync.dma_start(out=d_out[:], in_=gs_sb[:, 0:2])
                        else:
                            nc.sync.dma_start(out=gs_in[:], in_=gs_sb[:])

            if phases >= 4 and phases not in (6, 7, 8, 9):
                # gs_in is written by a DMA just above; the collective
                # trigger does not reliably wait for that write (graph-0
                # row garbage) — force completion first.
                tc.strict_bb_all_engine_barrier()
                nc.gpsimd.collective_compute(
                    "AllReduce", AX.add,
                    ins=[gs_in[:].opt()], outs=[gs_out[:].opt()],
                    replica_groups=[list(range(NC))],
                )

                # =========== final readout ===========
                with (
                    tc.tile_pool(name="fs", bufs=1) as fs,
                    tc.tile_pool(name="fp", bufs=1, space="PSUM") as fp,
                ):
                    gs2 = fs.tile([128, H2 + 1], F32, tag="gs2")
                    nc.sync.dma_start(out=gs2[:], in_=gs_out[:])
                    rcnt = fs.tile([128, 1], F32, tag="rcnt")
                    cntc = fs.tile([128, 1], F32, tag="cntc")
                    ctmp = fs.tile([128, 1], F32, tag="ctmp")
                    nc.vector.tensor_scalar(out=cntc[:],
                                            in0=gs2[:, H2 : H2 + 1],
                                            scalar1=1.0, scalar2=None,
                                            op0=AX.max)
                    nc.vector.reciprocal(out=rcnt[:], in_=cntc[:])
                    for _ in range(2):
                        nc.vector.tensor_mul(out=ctmp[:], in0=cntc[:],
                                             in1=rcnt[:])
                        nc.vector.tensor_scalar(out=ctmp[:], in0=ctmp[:],
                                                scalar1=-1.0, scalar2=2.0,
                                                op0=AX.mult, op1=AX.add)
                        nc.vector.tensor_mul(out=rcnt[:], in0=rcnt[:],
                                             in1=ctmp[:])
                    gr = fs.tile([128, H2], F32, tag="gr")
                    nc.vector.tensor_scalar(out=gr[:], in0=gs2[:, 0:H2],
                                            scalar1=rcnt[:], scalar2=None,
                                            op0=AX.mult)
                    grtp = fp.tile([128, H2], F32, tag="grtp", space="PSUM")
                    nc.tensor.transpose(out=grtp[:], in_=gr[:],
                                        identity=identf)
                    grt = fs.tile([128, H2], F32, tag="grt")
                    nc.vector.tensor_copy(out=grt[:], in_=grtp[:])
                    lps = fp.tile([128, 2], F32, tag="lps", space="PSUM")
                    nc.tensor.matmul(out=lps[:], lhsT=grt[:], rhs=Wc_sb,
                                     start=True, stop=True)
                    ologit = fs.tile([128, 2], F32, tag="ologit")
                    nc.vector.tensor_add(out=ologit[:], in0=lps[:], in1=bc_sb)
                    nc.sync.dma_start(out=d_out[:], in_=ologit[:])

    nc.compile()
    return nc


def make_in_maps(prep, wts, cfg):
    NC = cfg["NC"]
    maps = []
    for c in range(NC):
        constb = np.concatenate([
            prep["dl4"][c].astype(NPBF), wts["iota_bf"],
            prep["graph_loc"][c].astype(NPBF),
        ], axis=1)
        constf = np.concatenate([
            wts["w1c"], wts["b1c"], wts["Wc"], wts["bcrep"], wts["identf"],
            wts["W2a"], wts["W2b"], wts["b2rep"], prep["deg_all"][c],
            prep["padmask"],
        ], axis=1).astype(np.float32)
        consti = np.concatenate(
            [prep["src4"][m][c] for m in range(MEMB)], axis=1).astype(np.int32)
        maps.append(dict(constb=np.ascontiguousarray(constb),
                         constf=np.ascontiguousarray(constf),
                         consti=np.ascontiguousarray(consti),
                         dega=prep["dega"], degb=prep["degb"]))
    return maps


# --------------------------------------------------------------------------
# entry point
# --------------------------------------------------------------------------
def kernel(src, dst, graph_ids, W1, b1, W2, b2, Wc, bc):
    global LAST_RESULTS
    from concourse.bass_utils import run_bass_kernel_spmd

    cfg = FULL_CFG
    prep = host_prep(src, dst, graph_ids, cfg)
    wts = host_weights(W1, b1, W2, b2, Wc, bc)
    nc = build_program(prep, cfg)
    in_maps = make_in_maps(prep, wts, cfg)
    trace = bool(os.environ.get("GNN_TRACE"))
    res = run_bass_kernel_spmd(
        nc, in_maps, core_ids=list(range(cfg["NC"])), trace=trace,
    )
    LAST_RESULTS = res
    out = np.asarray(res.results[0]["out"])[: cfg["G"]]
    return out.astype(np.float32)
